# revision 22
# baseline (speedup 1.0000x reference)
"""Trainium2 Bass kernel for nn_AttentionalReadout (segment-softmax pooling).

v3 — worklog:
  v1 (508 us): PE-bound 93%, on-chip PE transposes + 81 MB HBM.
  v2 (312 us): dual host layouts (node-major + feature-major x in fp8e3m4),
      no transposes, 66 MB HBM.  Trace: PE 95% busy; pass-B pairs serialized
      by late DVE one-hot builds (298 ns each); tanh pays 260 ns ACT
      overhead per instruction; W1 streams fine.
  v3: - W1 matmul in fp8e4m3 DoubleRow (K=256 packed, 2x ALU rate);
        W1 host-scaled by 8 so its entries stay in e4m3 normal range,
        un-scaled inside tanh via the activation scale parameter.
      - pooling uses 4x PE column tiling (G_BLK=32): tile t accumulates
        into PSUM partitions 32*(t%4).. concurrently; host adds 4 slices.
      - one-hot E built per block with 2 large DVE tensor_tensors
        (EQ then MUL) instead of 1 tensor_scalar per tile.
      - gate PSUM is one [P, TB] bank per block, exp'd with a single ACT
        instruction; tanh batched per 8 tiles ([P, 1024] over 2 banks).
      - deeper software pipeline: pass B of block j is emitted after
        pass A of block j+1, giving the E builds a full pass-A window.

Algorithm (8-core SPMD, data-parallel over nodes):
  gate_i = tanh(x_i @ W1 + b1) @ W2     (b2 and the per-graph max cancel in
                                         the softmax; gate is bounded, so
                                         exp without the max shift is safe)
  out[g] = sum_i e_i x_i / sum_i e_i    with e_i = exp(gate_i)

Nodes are sharded at graph boundaries across cores; each core's node
stream is processed in blocks of TB 128-node tiles whose graphs fit in a
G_BLK window.  Per-block raw [128, 257] partials (feature sums + the
denominator column, 128/G_BLK col-group slices) are DMA'd out; host sums
partials across slices/blocks/cores and divides.
"""

import numpy as np

import concourse.bacc as bacc
import concourse.tile as tile
import concourse.mybir as mybir
from concourse.bass_utils import run_bass_kernel_spmd

P = 128            # nodes per tile (partition dim)
HDIM = 256         # node feature dim
HHID = 128         # gate MLP hidden dim
NUM_GRAPHS = 8192
N_CORES = 8
GROUP = 8          # tiles batched per tanh
XW = HDIM + 1      # output row: 256 feature sums + denominator column

_FP = mybir.dt.float32
_BF = mybir.dt.bfloat16
# e3m4 for x everywhere (max |x| ~5.5 << 15.5): the extra mantissa bit
# over e4m3 halves the pooled-output quantization error.  DoubleRow was
# tried for the W1 matmul (needs e4m3) and reverted: DR matmuls don't
# register as PE activity for the HAM clock gate, so the whole kernel
# ran at K=4/8 (1.2 GHz) for ~70% of its span.
_F8 = mybir.dt.float8e3
_NP_BF = mybir.dt.np(_BF)
_NP_F8 = mybir.dt.np(_F8)


def _plan(batch):
    """Choose node ranges per core and the uniform block geometry."""
    gpc = NUM_GRAPHS // N_CORES
    bounds = np.searchsorted(
        batch, np.arange(N_CORES + 1, dtype=np.int64) * gpc, side="left"
    ).astype(np.int64)
    t_need = max(1, int(np.ceil(np.diff(bounds).max() / P)))
    for tb, g_blk in [(24, 32), (16, 32), (8, 32), (32, 64), (16, 64)]:
        w = tb * P
        ok = True
        for c in range(N_CORES):
            s, e = int(bounds[c]), int(bounds[c + 1])
            nb = int(np.ceil(max(e - s, 0) / w))
            for j in range(nb):
                lo = s + j * w
                hi = min(lo + w, e)
                if hi <= lo:
                    continue
                if int(batch[hi - 1]) - int(batch[lo]) >= g_blk:
                    ok = False
                    break
            if not ok:
                break
        if ok:
            n_blocks = int(np.ceil(t_need / tb))
            return bounds, tb, g_blk, n_blocks, n_blocks * tb
    raise ValueError("no valid block plan for this batch vector")


def _build_program(T, TB, G_BLK, B):
    """Build the SPMD Bass program (identical across cores)."""
    n_cg = P // G_BLK          # column groups in the pooling matmul
    ng = TB // GROUP
    nc = bacc.Bacc("TRN2", target_bir_lowering=False, debug=False)
    # xn rows are exactly 256 B so every DMA packet is a full 2048 B burst
    # (a 257th ones-column would add an 8 B runt packet per partition run,
    # each costing as much DMA-engine time as a full burst)
    xn_d = nc.dram_tensor("xn", [P, T, HDIM], _F8, kind="ExternalInput")
    xt_d = nc.dram_tensor("xt", [P, T, 2, HHID], _F8, kind="ExternalInput")
    lidx_d = nc.dram_tensor("lidx", [P, T], _FP, kind="ExternalInput")
    consts_d = nc.dram_tensor("consts", [P, 1], _FP, kind="ExternalInput")
    # bf16 consts: [0:128] W1[:128,:], [128:256] W1[128:,:], [256] W2,
    # [257] ones, [258 : 258+TB*G_BLK] iota tiled TB times
    constsb_d = nc.dram_tensor("constsb", [P, 258 + TB * G_BLK], _BF,
                               kind="ExternalInput")
    out_d = nc.dram_tensor("out", [B, P, XW], _FP, kind="ExternalOutput")

    Tanh = mybir.ActivationFunctionType.Tanh
    Exp = mybir.ActivationFunctionType.Exp
    EQ = mybir.AluOpType.is_equal
    MUL = mybir.AluOpType.mult

    with tile.TileContext(nc) as tc:
        with (
            tc.tile_pool(name="const", bufs=1) as const_pool,
            tc.tile_pool(name="xn", bufs=3) as xn_pool,
            tc.tile_pool(name="xt", bufs=3) as xt_pool,
            tc.tile_pool(name="u", bufs=4) as u_pool,
            tc.tile_pool(name="es", bufs=2) as es_pool,
            tc.tile_pool(name="eq", bufs=2) as eq_pool,
            tc.tile_pool(name="E", bufs=2) as E_pool,
            tc.tile_pool(name="osb", bufs=2) as o_pool,
            tc.tile_pool(name="hp", bufs=2, space="PSUM") as h_pool,
            tc.tile_pool(name="gp", bufs=2, space="PSUM") as g_pool,
            tc.tile_pool(name="Up", bufs=2, space="PSUM") as U_pool,
        ):
            consts = const_pool.tile([P, 1], _FP)
            nc.sync.dma_start(consts[:], consts_d.ap()[:])
            constsb = const_pool.tile([P, 258 + TB * G_BLK], _BF)
            nc.sync.dma_start(constsb[:], constsb_d.ap()[:])
            lidx_sb = const_pool.tile([P, T], _FP)
            nc.gpsimd.dma_start(lidx_sb[:], lidx_d.ap()[:])
            b1c = consts[:, 0:1]
            w1lo = constsb[:, 0:HHID]
            w1hi = constsb[:, HHID:2 * HHID]
            w2c = constsb[:, 256:257]
            onec = constsb[:, 257:258]
            iota = constsb[:, 258:258 + TB * G_BLK]

            def emit_dma(j):
                xn_sb = xn_pool.tile([P, TB, HDIM], _F8)
                xt_sb = xt_pool.tile([P, TB, 2, HHID], _F8)
                nc.sync.dma_start(
                    xn_sb[:], xn_d.ap()[:, j * TB:(j + 1) * TB, :]
                )
                nc.gpsimd.dma_start(
                    xt_sb[:], xt_d.ap()[:, j * TB:(j + 1) * TB, :, :]
                )
                return xn_sb, xt_sb

            def emit_w1(j, g, xt_sb):
                # h^T for GROUP tiles: per 4 tiles (one PSUM bank) two plain
                # matmuls, W1 halves stationary, fp8 x^T streaming 512 cols
                h_ps = h_pool.tile([P, GROUP * HHID], _FP)
                half = GROUP // 2
                for sub in range(2):
                    a = g * GROUP + sub * half
                    for k, w1k in enumerate((w1lo, w1hi)):
                        nc.tensor.matmul(
                            h_ps[:, sub * half * HHID:(sub + 1) * half * HHID],
                            w1k, xt_sb[:, a:a + half, k, :],
                            start=(k == 0), stop=(k == 1),
                        )
                u_sb = u_pool.tile([P, GROUP * HHID], _BF)
                nc.scalar.activation(u_sb[:], h_ps[:], Tanh, bias=b1c)
                return u_sb

            def emit_gates(j, g, u_sb, gate_ps):
                for q in range(GROUP):
                    t = g * GROUP + q
                    nc.tensor.matmul(
                        gate_ps[:, t:t + 1],
                        u_sb[:, q * HHID:(q + 1) * HHID],
                        w2c, start=True, stop=True,
                    )

            def emit_exp_E(j, gate_ps):
                es = es_pool.tile([P, TB], _FP)
                nc.scalar.activation(es[:], gate_ps[:], Exp)
                eq_sb = eq_pool.tile([P, TB, G_BLK], _BF)
                nc.vector.tensor_tensor(
                    eq_sb[:],
                    lidx_sb[:, j * TB:(j + 1) * TB, None].to_broadcast(
                        [P, TB, G_BLK]),
                    iota.rearrange("p (t g) -> p t g", t=TB),
                    EQ,
                )
                E_sb = E_pool.tile([P, TB, G_BLK], _BF)
                nc.vector.tensor_tensor(
                    E_sb[:], eq_sb[:],
                    es[:, :, None].to_broadcast([P, TB, G_BLK]),
                    MUL,
                )
                return E_sb

            def emit_passA(j, xn_xt):
                xn_sb, xt_sb = xn_xt
                gate_ps = g_pool.tile([P, TB], _FP)
                us = []
                for g in range(ng):
                    us.append(emit_w1(j, g, xt_sb))
                    if g >= 1:
                        emit_gates(j, g - 1, us[g - 1], gate_ps)
                return xn_sb, gate_ps, us

            def emit_passA_tail(j, st):
                xn_sb, gate_ps, us = st
                emit_gates(j, ng - 1, us[ng - 1], gate_ps)
                E_sb = emit_exp_E(j, gate_ps)
                return xn_sb, E_sb

            def emit_passB(j, st):
                xn_sb, E_sb = st
                U_ps = U_pool.tile([P, XW], _FP)
                for t in range(TB):
                    grp = t % n_cg
                    sl = slice(grp * G_BLK, (grp + 1) * G_BLK)
                    nc.tensor.matmul(
                        U_ps[sl, 0:HDIM],
                        E_sb[:, t, :], xn_sb[:, t, :],
                        start=(t < n_cg), stop=(t >= TB - n_cg),
                        tile_position=(0, grp * G_BLK),
                    )
                    # denominator: 1-col matmul reusing the loaded E weights.
                    # start MUST be False: the feature matmul's start=True
                    # already marked this whole 2KB PSUM zero region
                    # pending-zero, so this chain's first write lands fresh;
                    # a second start=True would re-arm the pending-zero and
                    # make later feature accumulations overwrite (v4 bug).
                    nc.tensor.matmul(
                        U_ps[sl, HDIM:HDIM + 1],
                        E_sb[:, t, :], onec,
                        start=False, stop=(t >= TB - n_cg),
                        tile_position=(0, grp * G_BLK),
                        skip_group_check=True,
                    )
                out_sb = o_pool.tile([P, XW], _FP)
                nc.vector.tensor_copy(out_sb[:], U_ps[:])
                nc.sync.dma_start(out_d.ap()[j], out_sb[:])

            # deep pipeline: B(j-1) PE work lands between A(j) and A(j)'s
            # tail so the block-j E builds get a full pass-A window
            prev = None
            for j in range(B):
                xn_xt = emit_dma(j)
                st = emit_passA(j, xn_xt)
                if prev is not None:
                    emit_passB(j - 1, prev)
                prev = emit_passA_tail(j, st)
            emit_passB(B - 1, prev)

    nc.compile()
    return nc


def _prep_core(x8, batch, bounds, c, T, TB, G_BLK):
    """Per-core fp8 node-major / feature-major shards + lidx + block bases."""
    s, e = int(bounds[c]), int(bounds[c + 1])
    n = e - s
    xc = np.zeros((T * P, HDIM), dtype=_NP_F8)
    xc[:n] = x8[s:e]
    xn = np.ascontiguousarray(xc.reshape(T, P, HDIM).transpose(1, 0, 2))
    # xt[p, t, h, c] = x[node t*128+c, feat h*128+p]
    xt = np.ascontiguousarray(xc.reshape(T, P, 2, HHID).transpose(3, 0, 2, 1))

    w = TB * P
    Bn = T // TB
    g0 = np.zeros(Bn, dtype=np.int64)
    li = np.full(T * P, -1.0, dtype=np.float32)
    bl = batch[s:e]
    for j in range(Bn):
        lo = j * w
        hi = min(lo + w, n)
        if hi <= lo:
            g0[j] = int(batch[e - 1]) if n > 0 else 0
            continue
        g0[j] = int(bl[lo])
        li[lo:hi] = (bl[lo:hi] - g0[j]).astype(np.float32)
    lidx = np.ascontiguousarray(li.reshape(T, P).T)
    return xn, xt, lidx, g0


def _make_consts(W1, b1, W2, TB, G_BLK):
    consts = b1.reshape(P, 1).astype(np.float32)
    constsb = np.zeros((P, 258 + TB * G_BLK), dtype=_NP_BF)
    constsb[:, 0:HHID] = W1[:HHID, :].astype(_NP_BF)
    constsb[:, HHID:2 * HHID] = W1[HHID:, :].astype(_NP_BF)
    constsb[:, 256] = W2[:, 0].astype(_NP_BF)
    constsb[:, 257] = 1.0
    constsb[:, 258:] = np.tile(np.arange(G_BLK, dtype=np.float32), TB)[None, :]
    return consts, constsb


_CACHE = {}


def _get_program(T, TB, G_BLK, B):
    key = (T, TB, G_BLK, B)
    if key not in _CACHE:
        _CACHE[key] = _build_program(T, TB, G_BLK, B)
    return _CACHE[key]


def build_in_maps(x, W1, b1, W2, batch):
    """Host-side prep shared by kernel() and the timing harness."""
    batch = np.asarray(batch, dtype=np.int64)
    x = np.asarray(x, dtype=np.float32)
    bounds, TB, G_BLK, B, T = _plan(batch)
    consts, constsb = _make_consts(
        np.asarray(W1, dtype=np.float32),
        np.asarray(b1, dtype=np.float32),
        np.asarray(W2, dtype=np.float32),
        TB, G_BLK,
    )
    x8 = x.astype(_NP_F8)
    in_maps, g0s = [], []
    for c in range(N_CORES):
        xn, xt, lidx, g0 = _prep_core(x8, batch, bounds, c, T, TB, G_BLK)
        in_maps.append({
            "xn": xn, "xt": xt, "lidx": lidx,
            "consts": consts, "constsb": constsb,
        })
        g0s.append(g0)
    return in_maps, g0s, (T, TB, G_BLK, B)


def combine(results, g0s, G_BLK):
    """Sum per-block/per-col-group partials and normalize."""
    n_cg = P // G_BLK
    U = np.zeros((NUM_GRAPHS + G_BLK, HDIM), dtype=np.float64)
    S = np.zeros(NUM_GRAPHS + G_BLK, dtype=np.float64)
    for out_c, g0 in zip(results, g0s):
        for j in range(out_c.shape[0]):
            g = int(g0[j])
            blk = out_c[j, 0:G_BLK, :].astype(np.float64)
            for k in range(1, n_cg):
                blk += out_c[j, k * G_BLK:(k + 1) * G_BLK, :]
            U[g:g + G_BLK] += blk[:, :HDIM]
            S[g:g + G_BLK] += blk[:, HDIM]
    return (U[:NUM_GRAPHS] / (S[:NUM_GRAPHS, None] + 1e-16)).astype(np.float32)


def kernel(x, W1, b1, W2, b2, batch):
    in_maps, g0s, (T, TB, G_BLK, B) = build_in_maps(x, W1, b1, W2, batch)
    nc = _get_program(T, TB, G_BLK, B)
    res = run_bass_kernel_spmd(nc, in_maps, core_ids=list(range(N_CORES)))
    outs = [res.results[c]["out"] for c in range(N_CORES)]
    return combine(outs, g0s, G_BLK)
